# revision 48
# baseline (speedup 1.0000x reference)
"""NT-Xent loss kernel, v11 (47.86us cost-model, rel err 1.6e-3).

vs v2 (64.1us -> 52.1us cost-model, rel err 3.4e-3 -> 1.9e-4):
- host-side rotation makes every core's q-block rows 0-511, so zq /
  distq are SBUF views and zqT is a column view of zT (no extra DMAs,
  transposes, or copies for the q side);
- f32r dtype end to end for the z data path: sim/W matmuls at
  1 cycle/row (vs 4 for fp32) with near-fp32 precision, transposes at
  1.5 cycles/row, no bf16 conversion copies;
- one manual InstLoadActFuncSet of the ln+exp table set at program
  start; Ln and Exp then never reload tables (v2 thrashed 8x1283ns);
- norms: per-chunk square+reduce on DVE; nn=sqrt(norm2) and
  rn=rsqrt(norm2) via ln/exp on ACT (high_priority so the scheduler
  slots them between the big exp activations); z scaled in place per
  chunk: first tile x*=rn on DVE (two tiles for the last chunk),
  rest /=nn on GpSimd normalize_recip (splitting the scale across
  engines un-gates the per-chunk transpose chain, -2.5us);
- W = dist^T z_hat directly (32 f32r matmuls, no dist scaling),
  issued under tc.high_priority() so the scheduler slots them into PE
  gaps as soon as the last scaled z chunk lands (-1.6us tail);
- 8-chunk z DMA pipeline; chunk-pair transposes into [128,1024] PSUM
  tiles, copies split DVE (d=0) / ACT (d=1); single output DMA.

Known HW landmines (probed): tensor_tensor_reduce crashes the exec
unit; gpsimd tensor_scalar on f32r or int dtypes crashes; the
verifier requires f32r matmul inputs to be produced typed f32r
(normalize_recip and DVE/ACT copies qualify, bitcast writes do not);
transpose-mode matmul requires a permutation-matrix rhs.
"""

import numpy as np
from contextlib import ExitStack

N = 4096
D = 256
C = 100
B = 2048
N_CORES = 8
RPC = 512
NT = 32           # z row-tiles of 128
NCH = 8           # z DMA chunks (4 tiles each)
E_CONST = float(np.e)
MAGIC = 0x5F3759DF

_PROG = None


def _build_program():
    import concourse.bass as bass
    import concourse.tile as tile
    from concourse import bacc, mybir, masks

    f32 = mybir.dt.float32
    f32r = mybir.dt.float32r
    i32 = mybir.dt.int32
    MULT = mybir.AluOpType.mult
    ADD = mybir.AluOpType.add
    XOR = mybir.AluOpType.bitwise_xor
    SHR = mybir.AluOpType.arith_shift_right
    EXP = mybir.ActivationFunctionType.Exp
    LN = mybir.ActivationFunctionType.Ln
    AX = mybir.AxisListType.X

    nc = bacc.Bacc(
        "TRN2",
        target_bir_lowering=False,
        debug=False,
        enable_asserts=False,
        num_devices=N_CORES,
    )

    z = nc.dram_tensor("z", [N, D], f32r, kind="ExternalInput").ap()
    dist = nc.dram_tensor("dist", [B, C], f32r, kind="ExternalInput").ap()
    out = nc.dram_tensor("out", [128, 8], f32, kind="ExternalOutput").ap()

    with tile.TileContext(nc) as tc, ExitStack() as ctx:
        per = ctx.enter_context(tc.tile_pool(name="persist", bufs=1))

        # preload the ln+exp table set once; the insertion pass then adds
        # no further table loads for Ln or Exp anywhere in the program.
        nc.scalar.add_instruction(mybir.InstLoadActFuncSet(
            name=f"I-{nc.next_id()}", ins=[], outs=[], act_func_set_id=6))

        ident = per.tile([128, 128], f32)
        masks.make_identity(nc, ident[:])
        ident_r = per.tile([128, 128], f32r)
        nc.vector.tensor_copy(out=ident_r[:], in_=ident[:])

        zn = per.tile([128, NT * 256], f32r)
        zT = per.tile([128, 2 * 4096], f32r)
        dist_sb = per.tile([128, 16 * C], f32r)
        norm2 = per.tile([128, NT], f32)
        lnn = per.tile([128, NT], f32)
        nn = per.tile([128, NT], f32)
        rn = per.tile([128, NT], f32)
        wt_sb = per.tile([128, 256], f32)
        w_sb = per.tile([128, 2 * C], f32r)
        q_sb = per.tile([128, C], f32)
        junk = per.tile([128, C], f32)
        S_parts = per.tile([128, 16], f32)
        out_sb = per.tile([128, 8], f32)
        exp_scr = per.tile([128, 1024], f32)

        # ---------------- DMAs: 8 z chunks (4 tiles each), then dist --------
        for ch in range(NCH):
            nc.sync.dma_start(
                out=zn[:, ch * 1024:(ch + 1) * 1024].rearrange(
                    "p (t j) -> p t j", j=256),
                in_=z[ch * 512:(ch + 1) * 512, :].rearrange(
                    "(t p) j -> p t j", p=128),
            )
        nc.sync.dma_start(
            out=dist_sb[:].rearrange("p (t c) -> p t c", c=C),
            in_=dist.rearrange("(t p) c -> p t c", p=128),
        )

        # ------- norms: squares (DVE early / Pool late), reduce on DVE ----
        sqp = ctx.enter_context(tc.tile_pool(name="sqp", bufs=2))

        def norms_chunk(ch):
            sq = sqp.tile([128, 1024], f32, tag="sq")
            src_v = zn[:, ch * 1024:(ch + 1) * 1024].bitcast(f32)
            nc.vector.tensor_tensor(out=sq[:], in0=src_v, in1=src_v, op=MULT)
            nc.vector.tensor_reduce(
                out=norm2[:, ch * 4:(ch + 1) * 4],
                in_=sq[:].rearrange("p (t j) -> p t j", j=256),
                axis=AX, op=ADD,
            )

        def sqrt_chunk(ch):
            # nn = sqrt(norm2), rn = rsqrt(norm2) via ln/exp (set-6 tables)
            sl = slice(ch * 4, (ch + 1) * 4)
            with tc.high_priority():
                nc.scalar.activation(lnn[:, sl], norm2[:, sl], LN)
                nc.scalar.activation(nn[:, sl], lnn[:, sl], EXP, scale=0.5)
                nc.scalar.activation(rn[:, sl], lnn[:, sl], EXP, scale=-0.5)

        with tc.tile_pool(name="psum_tr", bufs=2, space="PSUM") as ptr, \
                tc.tile_pool(name="psum_c", bufs=2, space="PSUM") as pc:

            def transpose_pair(ch0):
                # 2 ptr tiles per chunk-pair: same-d for 8 z-tiles, so the
                # PSUM->SBUF copy is one [128,1024] op; d=0 on DVE, d=1 ACT.
                for d in range(2):
                    pt = ptr.tile([128, 1024], f32r, tag="tr")
                    for k in range(8):
                        t = ch0 * 4 + k
                        nc.tensor.transpose(
                            pt[:, k * 128:(k + 1) * 128],
                            zn[:, t * 256 + d * 128: t * 256 + d * 128 + 128],
                            ident_r[:],
                        )
                    dst = zT[:, d * 4096 + ch0 * 512: d * 4096 + (ch0 + 2) * 512]
                    if d == 0:
                        nc.vector.tensor_copy(out=dst, in_=pt[:])
                    else:
                        nc.scalar.copy(out=dst, in_=pt[:])

            def scale_chunk(ch):
                # z -> z_hat in place; split DVE (x*rn) / Pool (x/nn)
                for t in range(ch * 4, ch * 4 + 4):
                    if t % 4 == 0 or (ch == 7 and t % 4 == 1):
                        nc.vector.tensor_scalar(
                            out=zn[:, t * 256:(t + 1) * 256],
                            in0=zn[:, t * 256:(t + 1) * 256],
                            scalar1=rn[:, t:t + 1],
                            scalar2=None, op0=MULT)
                    else:
                        nc.gpsimd.normalize_recip(
                            out_ap=zn[:, t * 256:(t + 1) * 256],
                            in_ap=zn[:, t * 256:(t + 1) * 256].bitcast(f32),
                            denom_ap=nn[:, t:t + 1])

            def sims_group(j):
                # sim rows 0-511 x cols [j*1024,(j+1)*1024), exp row-sums
                for rt in range(4):
                    ps = pc.tile([128, 1024], f32, tag="sim")
                    for cc in range(2):
                        col0 = j * 1024 + cc * 512
                        for d in range(2):
                            nc.tensor.matmul(
                                ps[:, cc * 512:(cc + 1) * 512],
                                lhsT=zT[:, d * 4096 + rt * 128:
                                        d * 4096 + (rt + 1) * 128],
                                rhs=zT[:, d * 4096 + col0:
                                       d * 4096 + col0 + 512],
                                start=(d == 0), stop=(d == 1),
                            )
                    nc.scalar.activation(
                        exp_scr[:], ps[:], EXP,
                        accum_out=S_parts[:, rt * 4 + j: rt * 4 + j + 1],
                    )

            # chunk pipeline: norms -> rsqrt -> scale -> transposes,
            # sims for col-group j after chunks 2j, 2j+1 are transposed.
            # tile_set_cur_wait: floor each chunk's stage at its real DMA
            # arrival so the scheduler doesn't front-load late-chunk work
            # (its internal DMA model is optimistic) and head-of-line-block
            # the in-order engine queues.
            for ch in range(NCH):
                tc.tile_set_cur_wait((2.0 + 1.6 * ch) / 1000.0)
                norms_chunk(ch)
                sqrt_chunk(ch)
                scale_chunk(ch)
                if ch % 2 == 1:
                    transpose_pair(ch - 1)
                    sims_group(ch // 2)

        # ---------------- nominator: W = (rn*dist)^T z, P = zq_hat W -------
        with tc.tile_pool(name="psum_b", bufs=2, space="PSUM") as pb:
            wt_ps = pb.tile([128, 256], f32, tag="wt")
            with tc.high_priority():
                for t in range(NT):
                    nc.tensor.matmul(
                        wt_ps[0:C, :],
                        lhsT=dist_sb[:, (t % 16) * C:((t % 16) + 1) * C],
                        rhs=zn[:, t * 256:(t + 1) * 256],
                        start=(t == 0), stop=(t == NT - 1),
                    )
            with tc.high_priority():
                nc.vector.tensor_copy(out=wt_sb[0:C, :], in_=wt_ps[0:C, :])
                for d in range(2):
                    w_ps = pb.tile([128, 128], f32, tag="wq")
                    nc.tensor.transpose(
                        w_ps[:, 0:C],
                        wt_sb[0:C, d * 128:(d + 1) * 128],
                        ident[0:C, 0:C],
                    )
                    nc.vector.tensor_copy(out=w_sb[:, d * C:(d + 1) * C],
                                          in_=w_ps[:, 0:C])
            for rt in range(4):
                q_ps = pb.tile([128, 128], f32, tag="wq")
                for d in range(2):
                    nc.tensor.matmul(
                        q_ps[:, 0:C],
                        lhsT=zT[:, d * 4096 + rt * 128:
                                d * 4096 + (rt + 1) * 128],
                        rhs=w_sb[:, d * C:(d + 1) * C],
                        start=(d == 0), stop=(d == 1),
                    )
                nc.vector.tensor_copy(out=q_sb[:], in_=q_ps[:, 0:C])
                nc.vector.tensor_mul(junk[:], q_sb[:],
                                     dist_sb[:, rt * C:(rt + 1) * C].bitcast(f32))
                nc.vector.tensor_reduce(out=out_sb[:, 4 + rt:5 + rt],
                                        in_=junk[:], axis=AX, op=ADD)
            nc.vector.tensor_scalar(out=out_sb[:, 4:8], in0=out_sb[:, 4:8],
                                    scalar1=-1.0, scalar2=None, op0=ADD)

        for rt in range(4):
            nc.vector.tensor_reduce(
                out=out_sb[:, rt:rt + 1], in_=S_parts[:, rt * 4:rt * 4 + 4],
                axis=AX, op=ADD,
            )
        nc.vector.tensor_scalar(out=out_sb[:, 0:4], in0=out_sb[:, 0:4],
                                scalar1=-E_CONST, scalar2=None, op0=ADD)

        nc.sync.dma_start(out=out[:], in_=out_sb[:])

    nc.finalize()
    return nc


def _get_program():
    global _PROG
    if _PROG is None:
        _PROG = _build_program()
    return _PROG


def kernel(z_i, z_j, z_n, dist_labels):
    from concourse.bass_utils import run_bass_kernel_spmd

    nc = _get_program()

    z_full = np.ascontiguousarray(
        np.concatenate([z_i, z_j], axis=0), dtype=np.float32
    )
    dist = np.ascontiguousarray(dist_labels, dtype=np.float32)

    in_maps = []
    for c in range(N_CORES):
        r0 = c * RPC
        in_maps.append({
            "z": np.ascontiguousarray(np.roll(z_full, -r0, axis=0)),
            "dist": np.ascontiguousarray(np.roll(dist, -r0, axis=0)),
        })

    res = run_bass_kernel_spmd(nc, in_maps, list(range(N_CORES))).results

    S = np.empty(N, np.float64)
    P = np.empty(N, np.float64)
    for c in range(N_CORES):
        o = res[c]["out"]
        S[c * RPC:(c + 1) * RPC] = o[:, 0:4].T.reshape(RPC).astype(np.float64)
        P[c * RPC:(c + 1) * RPC] = o[:, 4:8].T.reshape(RPC).astype(np.float64)

    return np.float32((P / S).sum() / N)
